# revision 1
# baseline (speedup 1.0000x reference)
"""ConstituencyTreeLSTM Trainium2 kernel.

Strategy:
  - Data-parallel over the B=256 batch across 8 NeuronCores (32 rows/core).
  - The tree is a complete heap (node i has children 2i+1, 2i+2), so the
    sequential scan is reorganized into level-parallel phases:
      leaves (nodes 128..255) -> node 127 -> level 6 (63..126) -> ... -> root.
  - Everything on-device lives in a "feature-on-partitions, (node, ktile,
    batch) on free axis" layout, so matmul outputs (PSUM, [out_dim, rows])
    feed the next level's matmuls with no transposes.
  - h-path matmuls at deep levels (node level >= 3) run in fp8e4m3 with
    DoubleRow perf mode (2 k-tiles per instruction, 2x MAC throughput);
    shallow levels (4+2+1 nodes) stay bf16 for accuracy. x-path matmuls
    are bf16 everywhere (fp8 x fails the error budget). All weights are
    pre-scaled by 16 (exact in bf16, keeps the fp8 h-weights out of the
    e4m3 denormal range); the PSUM-evacuating activation applies
    scale=1/16.
  - The f-gate x-projection (x @ W_fx) is computed once per chunk into
    fx_t (PSUM -> Copy-activation); fL/fR accumulate only their h-path in
    PSUM and a DVE add applies fx_t, removing a duplicated 16-matmul
    group per 2-child chunk.
  - h of every level lives in SBUF level tiles (fp8 for levels 4..7, bf16
    for 1..3); parents read children h via stride-2 node slices
    (rearranged to [p, ktpair, node, batch] for DoubleRow).
  - c goes through DRAM (CL/CR, parity-split by parent index) for the big
    levels; SBUF level tiles for levels 4..1.
  - Weight/bias DMAs ride the Activation HWDGE queue so the first xt tile
    (SP queue) isn't stuck behind them; leaves only wait for the 1.5MB
    iou x-weight tile instead of all weights.
"""

import sys

sys.path.insert(0, "/opt/trn_rl_repo")

import numpy as np
import ml_dtypes

import concourse.bass as bass  # noqa: F401
import concourse.mybir as mybir
import concourse.tile as tile
from concourse import bacc
from concourse.bass_utils import run_bass_kernel_spmd

BF16 = ml_dtypes.bfloat16
FP8 = ml_dtypes.float8_e4m3
NCORES = 8
B, N, D = 256, 256, 512
BC = B // NCORES  # batch rows per core
NJ = 20  # 12 iou + 4 fL + 4 fR bias columns
WSCALE = 16.0

# x-path blocks: 12 iou j-tiles + 4 fx j-tiles, 4 k-tiles each. The iou js
# are ordered by kt-cohort (j = co, 4+co, 8+co) so the first DMA piece covers
# exactly what the first leaf cohort needs.
W_X_BLOCKS = [
    (kt, j) for co in range(4) for j in (co, 4 + co, 8 + co) for kt in range(4)
] + [(kt, j) for j in range(12, 16) for kt in range(4)]
WX_IDX = {p: i for i, p in enumerate(W_X_BLOCKS)}
NWX = len(W_X_BLOCKS)  # 64
NWX_IOU = 48  # iou blocks (cohort-ordered); the rest are the 16 fx blocks

# h-path blocks, DoubleRow-pair adjacent: per iou j: hL kt 0..4 then hR kt
# 0..4; per fL j: hL kt 0..4; per fR j: hR kt 0..4
W_H_BLOCKS = []
for j in range(12):
    W_H_BLOCKS += [("L", kt, j) for kt in range(4)]
    W_H_BLOCKS += [("R", kt, j) for kt in range(4)]
for j in range(12, 16):
    W_H_BLOCKS += [("L", kt, j) for kt in range(4)]
for j in range(16, 20):
    W_H_BLOCKS += [("R", kt, j) for kt in range(4)]
WH_IDX = {p: i for i, p in enumerate(W_H_BLOCKS)}
NWH = len(W_H_BLOCKS)  # 128

_compiled = {}


def _build_bass(reps=1):
    nc = bacc.Bacc("TRN2", target_bir_lowering=False, debug=False, num_devices=NCORES)

    f32 = mybir.dt.float32
    bf16 = mybir.dt.bfloat16
    fp8 = mybir.dt.float8e4
    DR = mybir.MatmulPerfMode.DoubleRow
    ACT = mybir.ActivationFunctionType

    xt = nc.dram_tensor("xt", [N, D, BC], bf16, kind="ExternalInput")
    xt8 = nc.dram_tensor("xt8", [N, D, BC], fp8, kind="ExternalInput")
    wx_d = nc.dram_tensor("wx", [NWX, 128, 128], bf16, kind="ExternalInput")
    wx8_d = nc.dram_tensor("wx8", [NWX, 128, 128], fp8, kind="ExternalInput")
    wh8_d = nc.dram_tensor("wh8", [NWH, 128, 128], fp8, kind="ExternalInput")
    b2_d = nc.dram_tensor("b2", [128, NJ], f32, kind="ExternalInput")
    bleaf_d = nc.dram_tensor("bleaf", [128, NJ], f32, kind="ExternalInput")
    b1_d = nc.dram_tensor("b1", [128, NJ], f32, kind="ExternalInput")

    c0t = nc.dram_tensor("c0t", [D, BC], f32, kind="ExternalOutput")
    h0t = nc.dram_tensor("h0t", [D, BC], f32, kind="ExternalOutput")

    # views: [partition, node, ktile, batch]
    xt_r = xt.ap().rearrange("n (kt p) b -> p n kt b", p=128)
    xt8_r = xt8.ap().rearrange("n (kt p) b -> p n kt b", p=128)
    c0t_r = c0t.ap().rearrange("(kt p) b -> p kt b", p=128)
    h0t_r = h0t.ap().rearrange("(kt p) b -> p kt b", p=128)

    with tile.TileContext(nc) as tc:
        import contextlib

        ctx = contextlib.ExitStack()
        with ctx:
            wpool = ctx.enter_context(tc.tile_pool(name="wpool", bufs=1))
            hpool = ctx.enter_context(tc.tile_pool(name="hpool", bufs=1))
            inpool = ctx.enter_context(tc.tile_pool(name="inpool", bufs=2))
            gpool = ctx.enter_context(tc.tile_pool(name="gpool", bufs=2))
            epool = ctx.enter_context(tc.tile_pool(name="epool", bufs=2))
            pspool = ctx.enter_context(tc.tile_pool(name="ps", bufs=6, space="PSUM"))
            fxpool = ctx.enter_context(tc.tile_pool(name="fxps", bufs=2, space="PSUM"))

            # --- weights / biases ---------------------------------------
            # All weight DMAs ride the Pool (gpsimd) SWDGE queue in 16-block
            # pieces: small pieces interleave with the SP-queue xt prefetches
            # on the DMA engines instead of starving them, and the idle Pool
            # sequencer absorbs the issue cost. The leaf phase only needs the
            # wx_iou pieces (first on the queue) + bleaf (SP, tiny).
            wx_iou_sb = wpool.tile([128, NWX_IOU, 128], bf16, name="wxiou")
            wx_f_sb = wpool.tile([128, NWX - NWX_IOU, 128], bf16, name="wxf")
            wx8_sb = wpool.tile([128, NWX, 128], fp8, name="wx8")
            wh8_sb = wpool.tile([128, NWH, 128], fp8, name="wh8")
            b2_sb = wpool.tile([128, NJ], f32, name="b2sb")
            bleaf_sb = wpool.tile([128, NJ], f32, name="bleafsb")
            b1_sb = wpool.tile([128, NJ], f32, name="b1sb")

            wx_r = wx_d.ap().rearrange("blk p c -> p blk c")
            wx8_r = wx8_d.ap().rearrange("blk p c -> p blk c")
            wh8_r = wh8_d.ap().rearrange("blk p c -> p blk c")
            nc.sync.dma_start(out=bleaf_sb[:], in_=bleaf_d.ap()[:])
            # order: bf16 iou x-blocks (leaves, cohort-piece first), h weights
            # (node 127 / L6), fp8 x-blocks (L6/L5), bf16 fx + biases.
            for s in range(0, NWX_IOU, 12):
                nc.gpsimd.dma_start(
                    out=wx_iou_sb[:, s : s + 12, :], in_=wx_r[:, s : s + 12, :]
                )
            for s in range(0, NWH, 16):
                nc.gpsimd.dma_start(
                    out=wh8_sb[:, s : s + 16, :], in_=wh8_r[:, s : s + 16, :]
                )
            for s in range(0, NWX, 32):
                nc.gpsimd.dma_start(
                    out=wx8_sb[:, s : s + 32, :], in_=wx8_r[:, s : s + 32, :]
                )
            nc.gpsimd.dma_start(out=wx_f_sb[:], in_=wx_r[:, NWX_IOU:, :])
            nc.gpsimd.dma_start(out=b2_sb[:], in_=b2_d.ap()[:])
            nc.gpsimd.dma_start(out=b1_sb[:], in_=b1_d.ap()[:])

            def wx_ap(kt, j):
                if j < 12:
                    return wx_iou_sb[:, WX_IDX[(kt, j)], :]
                return wx_f_sb[:, WX_IDX[(kt, j)] - NWX_IOU, :]

            def process(
                nodes,
                has_l,
                has_r,
                bias_sb,
                child_h,  # list[(tile, base)] — 1 (plain fp8 h) or 2 (h8+res)
                out_h,  # list[(tile, base)] or None (root)
                child_c=None,  # (tile, base_node) -> children c from SBUF
                out_c=None,  # (tile, base_node) -> write c to SBUF
                chunk_starts=None,  # custom chunk order (e.g. L6 defers 63..78)
                x8=False,  # x-path in fp8 DoubleRow (levels 6 and 5)
            ):
                """Compute (c, h) for `nodes` (a range), all at one depth."""
                to_out = out_h is None
                for a in chunk_starts or range(nodes.start, nodes.stop, 16):
                    b_ = min(a + 16, nodes.stop)
                    k = b_ - a  # nodes in this chunk
                    dt_g = f32 if to_out else bf16

                    if x8:
                        xt_t = inpool.tile([128, k, 4, BC], fp8, name="xt8_t")
                        nc.sync.dma_start(out=xt_t[:], in_=xt8_r[:, a:b_, :, :])
                    else:
                        xt_t = inpool.tile([128, k, 4, BC], bf16, name="xt_t")
                        nc.sync.dma_start(out=xt_t[:], in_=xt_r[:, a:b_, :, :])

                    def x_group(ps, j, stop_at_end):
                        """x-path matmuls of j into ps (starts the group)."""
                        if x8:
                            i0 = WX_IDX[(0, j)]
                            for m, kk in enumerate((0, 2)):
                                nc.tensor.matmul(
                                    ps[:],
                                    wx8_sb[:, i0 + kk : i0 + kk + 2, :],
                                    xt_t[:, :, kk : kk + 2, :].rearrange(
                                        "p n kt b -> p kt n b"
                                    ),
                                    start=(m == 0),
                                    stop=(m == 1 and stop_at_end),
                                    perf_mode=DR,
                                )
                        else:
                            for kk in range(4):
                                nc.tensor.matmul(
                                    ps[:],
                                    wx_ap(kk, j),
                                    xt_t[:, :, kk, :],
                                    start=(kk == 0),
                                    stop=(kk == 3 and stop_at_end),
                                )
                    if child_c is not None:
                        cc_t, cc_base = child_c
                        cs0 = 2 * a + 1 - cc_base
                        if has_l:
                            if k == 1:
                                cl_t = cc_t[:, cs0 : cs0 + 1, :, :]
                            else:
                                cl_t = cc_t[:, cs0 : cs0 + 2 * k - 1 : 2, :, :]
                        if has_r:
                            if k == 1:
                                cr_t = cc_t[:, cs0 + 1 : cs0 + 2, :, :]
                            else:
                                cr_t = cc_t[:, cs0 + 1 : cs0 + 2 * k : 2, :, :]

                    if child_h is not None:
                        ch_base = child_h[0][1]
                        sl0 = 2 * a + 1 - ch_base

                        def nsl(off):
                            s0 = sl0 + off
                            if k == 1:
                                return slice(s0, s0 + 1)
                            return slice(s0, s0 + 2 * k - 1, 2)

                        def chs(ct, kta, ktb, off):
                            """children h, kt pair, as [p, kt, node, b]."""
                            return ct[:, nsl(off), kta:ktb, :].rearrange(
                                "p n kt b -> p kt n b"
                            )

                    g_i = gpool.tile([128, k, 4, BC], dt_g, name="g_i")
                    g_o = gpool.tile([128, k, 4, BC], dt_g, name="g_o")
                    g_u = gpool.tile([128, k, 4, BC], dt_g, name="g_u")
                    if has_l:
                        g_fl = gpool.tile([128, k, 4, BC], dt_g, name="g_fl", bufs=1)
                    if has_r:
                        g_fr = gpool.tile([128, k, 4, BC], dt_g, name="g_fr", bufs=1)
                    have_f = has_l or has_r
                    if have_f:
                        fx_t = gpool.tile([128, k, 4, BC], dt_g, name="fx_t")

                    def h_chain(ps, j, started):
                        """accumulate the h-path of j into ps (fp8 DoubleRow);
                        2-component child h (h8 + residual) runs two passes."""
                        sides = []
                        if has_l and j < 16:
                            sides.append(("L", 0))
                        if has_r and (j < 12 or 16 <= j):
                            sides.append(("R", 1))
                        insts = []
                        for side, off in sides:
                            i0 = WH_IDX[(side, 0, j)]
                            for ct, _ in child_h:
                                insts.append(
                                    (wh8_sb[:, i0 : i0 + 2, :], chs(ct, 0, 2, off))
                                )
                                insts.append(
                                    (wh8_sb[:, i0 + 2 : i0 + 4, :], chs(ct, 2, 4, off))
                                )
                        for m, (w_ap, rhs) in enumerate(insts):
                            nc.tensor.matmul(
                                ps[:],
                                w_ap,
                                rhs,
                                start=(not started and m == 0),
                                stop=(m == len(insts) - 1),
                                perf_mode=DR,
                            )

                    # --- kt cohorts: js {kt, 4+kt, 8+kt, 12+kt, 16+kt}, then
                    # that kt's elementwise. Each kt chain completes
                    # independently, so the next level's matmuls only wait for
                    # the last cohort instead of the whole chunk, and DVE/Act
                    # work overlaps later cohorts' matmuls.
                    if out_c is not None:
                        oc_t, oc_base = out_c
                        c_t = oc_t[:, a - oc_base : b_ - oc_base, :, :]
                    else:
                        c_t = epool.tile([128, k, 4, BC], dt_g, name="c_t")[:]
                    if have_f:
                        acc = epool.tile([128, k, 4, BC], dt_g, name="acc", bufs=1)
                        m2f = epool.tile([128, k, 4, BC], dt_g, name="m2f", bufs=1)
                        if has_l and has_r:
                            m3f = epool.tile([128, k, 4, BC], dt_g, name="m3f", bufs=1)
                        tmpf_l = gpool.tile([128, k, 4, BC], dt_g, name="tmpf_l", bufs=1)
                        tmpf_r = gpool.tile([128, k, 4, BC], dt_g, name="tmpf_r", bufs=1)
                    tc_t = epool.tile([128, k, 4, BC], dt_g, name="tc_t", bufs=1)
                    if to_out:
                        h_t = epool.tile([128, k, 4, BC], dt_g, name="h_t")
                    if out_h is not None and len(out_h) == 2:
                        hbf = epool.tile([128, k, 4, BC], bf16, name="hbf", bufs=1)
                        hsl = slice(a - out_h[0][1], b_ - out_h[0][1])

                    for kt in range(4):
                        # this cohort's fx j-tile: PSUM -> SBUF via Act Copy
                        # (walrus forbids a DVE TensorTensor on two PSUM aps)
                        if have_f:
                            ps_fx = fxpool.tile([128, k, BC], f32, name="ps_fx")
                            x_group(ps_fx, 12 + kt, stop_at_end=True)
                            nc.scalar.activation(
                                out=fx_t[:, :, kt, :], in_=ps_fx[:], func=ACT.Copy
                            )
                        cjs = [kt, 4 + kt, 8 + kt]
                        if has_l:
                            cjs.append(12 + kt)
                        if has_r:
                            cjs.append(16 + kt)
                        for j in cjs:
                            ps = pspool.tile([128, k, BC], f32, name="ps")
                            if j < 12:
                                x_group(ps, j, stop_at_end=(child_h is None))
                                if child_h is not None:
                                    h_chain(ps, j, started=True)
                                func = ACT.Tanh if 8 <= j else ACT.Sigmoid
                                dst = (g_i, g_o, g_u)[j // 4][:, :, kt, :]
                                nc.scalar.activation(
                                    out=dst,
                                    in_=ps[:],
                                    func=func,
                                    bias=bias_sb[:, j : j + 1],
                                    scale=1.0 / WSCALE,
                                )
                            else:
                                # f gate: h-path in PSUM + fx psum via DVE
                                h_chain(ps, j, started=False)
                                tmp = tmpf_l if j < 16 else tmpf_r
                                tslice = tmp[:, :, kt, :]
                                nc.vector.tensor_add(
                                    tslice, ps[:], fx_t[:, :, kt, :]
                                )
                                g_f = g_fl if j < 16 else g_fr
                                nc.scalar.activation(
                                    out=g_f[:, :, kt, :],
                                    in_=tslice,
                                    func=ACT.Sigmoid,
                                    bias=bias_sb[:, j : j + 1],
                                    scale=1.0 / WSCALE,
                                )

                        # --- elementwise for this kt ---
                        ct_s = c_t[:, :, kt, :]
                        ei = g_i[:, :, kt, :]
                        eu = g_u[:, :, kt, :]
                        eo = g_o[:, :, kt, :]
                        if not have_f:
                            nc.vector.tensor_mul(ct_s, ei, eu)
                        else:
                            accs = acc[:, :, kt, :]
                            nc.vector.tensor_mul(accs, ei, eu)
                            m2s = m2f[:, :, kt, :]
                            if has_l:
                                nc.vector.tensor_mul(
                                    m2s, g_fl[:, :, kt, :], cl_t[:, :, kt, :]
                                )
                            else:
                                nc.vector.tensor_mul(
                                    m2s, g_fr[:, :, kt, :], cr_t[:, :, kt, :]
                                )
                            if has_l and has_r:
                                nc.vector.tensor_add(accs, accs, m2s)
                                m3s = m3f[:, :, kt, :]
                                nc.vector.tensor_mul(
                                    m3s, g_fr[:, :, kt, :], cr_t[:, :, kt, :]
                                )
                                nc.vector.tensor_add(ct_s, accs, m3s)
                            else:
                                nc.vector.tensor_add(ct_s, accs, m2s)
                        tcs = tc_t[:, :, kt, :]
                        nc.scalar.activation(out=tcs, in_=ct_s, func=ACT.Tanh)
                        if to_out:
                            nc.vector.tensor_mul(h_t[:, :, kt, :], eo, tcs)
                        elif len(out_h) == 1:
                            oh_t, oh_base = out_h[0]
                            nc.vector.tensor_mul(
                                oh_t[:, a - oh_base : b_ - oh_base, kt, :], eo, tcs
                            )
                        else:
                            # split-h storage: h8 = fp8(h), r8 = fp8(h - h8);
                            # two DoubleRow passes at the parent recover ~bf16
                            # precision from fp8-weight matmuls.
                            hbs = hbf[:, :, kt, :]
                            nc.vector.tensor_mul(hbs, eo, tcs)
                            h8s = out_h[0][0][:, hsl, kt, :]
                            nc.vector.tensor_copy(h8s, hbs)
                            nc.vector.tensor_sub(
                                out_h[1][0][:, hsl, kt, :], hbs, h8s
                            )

                    if to_out:
                        nc.sync.dma_start(out=c0t_r[:], in_=c_t[:, 0, :, :])
                        nc.sync.dma_start(out=h0t_r[:], in_=h_t[:, 0, :, :])

            # h storage: plain fp8 for levels 4..7; split fp8 (h8 + residual)
            # for levels 1..3, whose parents need ~bf16 h precision.
            # c lives entirely in SBUF: fp8 at level 7 (bounded |i*u| < 1,
            # 7 forget-gates of attenuation), bf16 below.
            H_SPLIT_LVLS = (3, 2, 1)

            for _rep in range(reps):
                leafc_h = hpool.tile([128, 129, 4, BC], fp8, name="h_leafc")
                leafc_c = hpool.tile([128, 129, 4, BC], fp8, name="c_leafc")
                lvl_h = {7: [(leafc_h, 127)]}
                lvl_c = {7: (leafc_c, 127)}
                for lvl in range(6, 0, -1):
                    base = 2**lvl - 1
                    if lvl in H_SPLIT_LVLS:
                        t8 = hpool.tile([128, 2**lvl, 4, BC], fp8, name=f"h_{lvl}")
                        r8 = hpool.tile([128, 2**lvl, 4, BC], fp8, name=f"hr_{lvl}")
                        lvl_h[lvl] = [(t8, base), (r8, base)]
                    else:
                        t = hpool.tile([128, 2**lvl, 4, BC], fp8, name=f"h_{lvl}")
                        lvl_h[lvl] = [(t, base)]
                    t = hpool.tile([128, 2**lvl, 4, BC], bf16, name=f"c_{lvl}")
                    lvl_c[lvl] = (t, base)

                # leaves: nodes 128..255 (no children)
                process(
                    range(128, 256), False, False, bleaf_sb, None, lvl_h[7],
                    out_c=lvl_c[7],
                )
                # node 127: left child only (node 255, leafc slot 128)
                process(
                    range(127, 128), True, False, b1_sb, lvl_h[7], lvl_h[7],
                    child_c=lvl_c[7], out_c=lvl_c[7],
                )
                # levels 6..1: two children each. L6's first chunk (nodes
                # 63..78) needs node 127's h, so it runs last — the other
                # three L6 chunks (pure-leaf children) hide node 127's serial
                # latency.
                for lvl in range(6, 0, -1):
                    process(
                        range(2**lvl - 1, 2 ** (lvl + 1) - 1),
                        True,
                        True,
                        b2_sb,
                        lvl_h[lvl + 1] if lvl < 6 else lvl_h[7],
                        lvl_h[lvl],
                        child_c=lvl_c[lvl + 1] if lvl < 6 else lvl_c[7],
                        out_c=lvl_c[lvl],
                        chunk_starts=[79, 95, 111, 63] if lvl == 6 else None,
                        x8=(lvl in (6, 5)),
                    )
                # root
                process(
                    range(0, 1), True, True, b2_sb, lvl_h[1], None,
                    child_c=lvl_c[1],
                )

    nc.compile()
    return nc


def _expected_tree():
    left = np.array([2 * i + 1 if 2 * i + 1 < N else 0 for i in range(N)], np.int32)
    right = np.array([2 * i + 2 if 2 * i + 2 < N else 0 for i in range(N)], np.int32)
    nch = np.array(
        [int(2 * i + 1 < N) + int(2 * i + 2 < N) for i in range(N)], np.int32
    )
    return left, right, nch


def pack_w(W_ioux, W_fx, W_iouhL, W_fhL, W_iouhR, W_fhR):
    """Returns (wx bf16, wx8 fp8 [NWX,128,128], wh8 fp8 [NWH,128,128])."""
    s = WSCALE
    WxT = np.asarray(W_ioux, np.float32).T * s  # [512, 1536]
    WfxT = np.asarray(W_fx, np.float32).T * s  # [512, 512]
    wx = np.empty((NWX, 128, 128), np.float32)
    for i, (kt, j) in enumerate(W_X_BLOCKS):
        src = WxT if j < 12 else WfxT
        jj = j if j < 12 else j - 12
        wx[i] = src[kt * 128 : (kt + 1) * 128, jj * 128 : (jj + 1) * 128]

    WhT = {
        "L": (np.asarray(W_iouhL, np.float32).T * s,
              np.asarray(W_fhL, np.float32).T * s),
        "R": (np.asarray(W_iouhR, np.float32).T * s,
              np.asarray(W_fhR, np.float32).T * s),
    }
    wh = np.empty((NWH, 128, 128), np.float32)
    for i, (side, kt, j) in enumerate(W_H_BLOCKS):
        iou_m, f_m = WhT[side]
        if j < 12:
            wh[i] = iou_m[kt * 128 : (kt + 1) * 128, j * 128 : (j + 1) * 128]
        else:
            jj = (j - 12) if j < 16 else (j - 16)
            wh[i] = f_m[kt * 128 : (kt + 1) * 128, jj * 128 : (jj + 1) * 128]

    return (
        np.ascontiguousarray(wx).astype(BF16),
        np.ascontiguousarray(wx).astype(FP8),
        np.ascontiguousarray(wh).astype(FP8),
    )


def pack_biases(b_ioux, b_iouh, b_iouhL, b_iouhR, b_fx, b_fhL, b_fhR):
    def pack(vec):
        return np.ascontiguousarray(np.asarray(vec, np.float32).reshape(NJ, 128).T)

    z = np.zeros(512, np.float32)
    b2 = pack(np.concatenate([b_ioux + b_iouhL + b_iouhR, b_fx + b_fhL, b_fx + b_fhR]))
    bleaf = pack(np.concatenate([b_ioux + b_iouh, z, z]))
    b1 = pack(np.concatenate([b_ioux + b_iouhL, b_fx + b_fhL, z]))
    return b2, bleaf, b1


def kernel(
    inputs,
    W_ioux, b_ioux, W_iouh, b_iouh, W_iouhL, b_iouhL, W_iouhR, b_iouhR,
    W_fx, b_fx, W_fh, b_fh, W_fhL, b_fhL, W_fhR, b_fhR,
    left_idx, right_idx, num_children,
):
    el, er, en = _expected_tree()
    assert np.array_equal(np.asarray(left_idx), el), "unexpected tree structure"
    assert np.array_equal(np.asarray(right_idx), er), "unexpected tree structure"
    assert np.array_equal(np.asarray(num_children), en), "unexpected tree structure"

    inputs = np.asarray(inputs, np.float32)

    wx, wx8, wh8 = pack_w(W_ioux, W_fx, W_iouhL, W_fhL, W_iouhR, W_fhR)
    b_args = [
        np.asarray(v, np.float32)
        for v in (b_ioux, b_iouh, b_iouhL, b_iouhR, b_fx, b_fhL, b_fhR)
    ]
    b2, bleaf, b1 = pack_biases(*b_args)

    if "nc" not in _compiled:
        _compiled["nc"] = _build_bass()
    nc = _compiled["nc"]

    in_maps = []
    for c in range(NCORES):
        xc = inputs[c * BC : (c + 1) * BC]  # [BC, N, D]
        xt_c = np.ascontiguousarray(xc.transpose(1, 2, 0))  # [N, D, BC] f32
        in_maps.append(
            {"xt": xt_c.astype(BF16), "xt8": xt_c.astype(FP8),
             "wx": wx, "wx8": wx8, "wh8": wh8,
             "b2": b2, "bleaf": bleaf, "b1": b1}
        )

    res = run_bass_kernel_spmd(
        nc, in_maps, core_ids=list(range(NCORES)), trace=bool(_compiled.get("trace"))
    )
    _compiled["last_res"] = res

    c_full = np.empty((B, D), np.float32)
    h_full = np.empty((B, D), np.float32)
    for c in range(NCORES):
        c_full[c * BC : (c + 1) * BC] = res.results[c]["c0t"].T
        h_full[c * BC : (c + 1) * BC] = res.results[c]["h0t"].T
    return c_full, h_full



# revision 2
# speedup vs baseline: 1008.3757x; 1008.3757x over previous
"""ConstituencyTreeLSTM Trainium2 kernel, v2.

Changes vs v1 baseline:
  - Leaf x-path in fp8 DoubleRow (accuracy-validated: rel ~1.46e-2 < 2e-2).
  - 32-node super-chunks: per-j activations merged along the node axis
    (same output slice -> same bias), tanh(c)/h/elementwise merged across
    the 4 output slices. ~240 Act instructions instead of ~460.
  - fx evacuated by DVE (per-slice psum->SBUF copies), f-gate preact =
    DVE add (psum + fx_t) -> tmp SBUF, act reads SBUF.
  - Partition-major DRAM packing for weights and inputs: every DMA is
    contiguous per partition (KB-sized descriptor runs, not 32-64B).
  - Tail (nodes 0..30 + 127) x-inputs SBUF-resident, loaded once.
  - Tile reuse: tanh(c) overwrites g_u, mul scratch overwrites g_i,
    split-h intermediate overwrites g_fl.
"""

import sys

sys.path.insert(0, "/opt/trn_rl_repo")

import numpy as np
import ml_dtypes

import concourse.bass as bass  # noqa: F401
import concourse.mybir as mybir
import concourse.tile as tile
from concourse import bacc
from concourse.bass_utils import run_bass_kernel_spmd

BF16 = ml_dtypes.bfloat16
FP8 = ml_dtypes.float8_e4m3
NCORES = 8
B, N, D = 256, 256, 512
BC = B // NCORES
NJ = 20
WSCALE = 16.0

# x-path blocks: 12 iou j-tiles + 4 fx j-tiles, 4 k-tiles each; iou js
# cohort-ordered (j = co, 4+co, 8+co) so the first DMA piece covers the
# first j-groups processed.
W_X_BLOCKS = [
    (kt, j) for co in range(4) for j in (co, 4 + co, 8 + co) for kt in range(4)
] + [(kt, j) for j in range(12, 16) for kt in range(4)]
WX_IDX = {p: i for i, p in enumerate(W_X_BLOCKS)}
NWX = len(W_X_BLOCKS)  # 64
NWX_IOU = 48

W_H_BLOCKS = []
for j in range(12):
    W_H_BLOCKS += [("L", kt, j) for kt in range(4)]
    W_H_BLOCKS += [("R", kt, j) for kt in range(4)]
for j in range(12, 16):
    W_H_BLOCKS += [("L", kt, j) for kt in range(4)]
for j in range(16, 20):
    W_H_BLOCKS += [("R", kt, j) for kt in range(4)]
WH_IDX = {p: i for i, p in enumerate(W_H_BLOCKS)}
NWH = len(W_H_BLOCKS)  # 128

# tail nodes resident in SBUF: 0..30 plus 127 at position 31
TAIL_POS = {n: n for n in range(31)}
TAIL_POS[127] = 31

IOU_ORDER = [0, 4, 8, 1, 5, 9, 2, 6, 10, 3, 7, 11]  # cohort order (DMA-friendly)

_compiled = {}


def _build_bass(reps=1):
    nc = bacc.Bacc("TRN2", target_bir_lowering=False, debug=False, num_devices=NCORES)

    f32 = mybir.dt.float32
    bf16 = mybir.dt.bfloat16
    fp8 = mybir.dt.float8e4
    DR = mybir.MatmulPerfMode.DoubleRow
    ACT = mybir.ActivationFunctionType

    # partition-major DRAM layouts (host pre-packed)
    xt8_d = nc.dram_tensor("xt8", [128, N, 4, BC], fp8, kind="ExternalInput")
    xtt_d = nc.dram_tensor("xtt", [128, 32, 4, BC], bf16, kind="ExternalInput")
    ident_d = nc.dram_tensor("ident", [128, 128], bf16, kind="ExternalInput")
    wx_d = nc.dram_tensor("wx", [128, NWX, 128], bf16, kind="ExternalInput")
    wx8_d = nc.dram_tensor("wx8", [128, NWX, 128], fp8, kind="ExternalInput")
    wh8_d = nc.dram_tensor("wh8", [128, NWH, 128], fp8, kind="ExternalInput")
    b2_d = nc.dram_tensor("b2", [128, NJ], f32, kind="ExternalInput")
    bleaf_d = nc.dram_tensor("bleaf", [128, NJ], f32, kind="ExternalInput")
    b1_d = nc.dram_tensor("b1", [128, NJ], f32, kind="ExternalInput")

    c0t = nc.dram_tensor("c0t", [D, BC], f32, kind="ExternalOutput")
    h0t = nc.dram_tensor("h0t", [D, BC], f32, kind="ExternalOutput")

    xt8_r = xt8_d.ap()
    c0t_r = c0t.ap().rearrange("(kt p) b -> p kt b", p=128)
    h0t_r = h0t.ap().rearrange("(kt p) b -> p kt b", p=128)

    with tile.TileContext(nc) as tc:
        import contextlib

        ctx = contextlib.ExitStack()
        with ctx:
            wpool = ctx.enter_context(tc.tile_pool(name="wpool", bufs=1))
            hpool = ctx.enter_context(tc.tile_pool(name="hpool", bufs=1))
            inpool = ctx.enter_context(tc.tile_pool(name="inpool", bufs=2))
            gpool = ctx.enter_context(tc.tile_pool(name="gpool", bufs=1))
            epool = ctx.enter_context(tc.tile_pool(name="epool", bufs=1))
            pspool = ctx.enter_context(tc.tile_pool(name="ps", bufs=4, space="PSUM"))

            # --- weights / biases / tail inputs (one-time) ----------------
            wx_sb = wpool.tile([128, NWX, 128], bf16, name="wxsb")
            wx8_sb = wpool.tile([128, NWX, 128], fp8, name="wx8")
            wh8_sb = wpool.tile([128, NWH, 128], fp8, name="wh8")
            b2_sb = wpool.tile([128, NJ], f32, name="b2sb")
            bleaf_sb = wpool.tile([128, NJ], f32, name="bleafsb")
            b1_sb = wpool.tile([128, NJ], f32, name="b1sb")
            xtt_sb = wpool.tile([128, 32, 4, BC], bf16, name="xttsb")
            ident_sb = wpool.tile([128, 128], bf16, name="identsb")
            b2s_sb = wpool.tile([128, NJ], bf16, name="b2ssb")
            b1s_sb = wpool.tile([128, NJ], bf16, name="b1ssb")

            nc.sync.dma_start(out=bleaf_sb[:], in_=bleaf_d.ap()[:])
            nc.sync.dma_start(out=b2_sb[:], in_=b2_d.ap()[:])
            nc.sync.dma_start(out=b1_sb[:], in_=b1_d.ap()[:])
            nc.sync.dma_start(out=ident_sb[:], in_=ident_d.ap()[:])
            # x16-scaled bf16 biases for the identity-matmul bias injection
            nc.vector.tensor_single_scalar(
                b2s_sb[:], b2_sb[:], WSCALE, mybir.AluOpType.mult
            )
            nc.vector.tensor_single_scalar(
                b1s_sb[:], b1_sb[:], WSCALE, mybir.AluOpType.mult
            )
            # order: fp8 iou x-blocks (leaves first), h weights (127/L6),
            # fp8 fx blocks (L6/L5), bf16 wx + biases + tail x.
            for s in range(0, NWX_IOU, 12):
                nc.gpsimd.dma_start(
                    out=wx8_sb[:, s : s + 12, :], in_=wx8_d.ap()[:, s : s + 12, :]
                )
            nc.gpsimd.dma_start(out=xtt_sb[:], in_=xtt_d.ap()[:])
            for s in range(0, NWH, 32):
                nc.gpsimd.dma_start(
                    out=wh8_sb[:, s : s + 32, :], in_=wh8_d.ap()[:, s : s + 32, :]
                )
            nc.gpsimd.dma_start(
                out=wx8_sb[:, NWX_IOU:, :], in_=wx8_d.ap()[:, NWX_IOU:, :]
            )
            nc.gpsimd.dma_start(out=wx_sb[:], in_=wx_d.ap()[:])

            def process(
                nodes,
                has_l,
                has_r,
                bias_sb,
                child_h,  # list[(tile, base)] or None
                out_h,  # list[(tile, base)] or None (root)
                child_c=None,
                out_c=None,
                x8=False,
                bias_s_sb=None,
            ):
                a, b_ = nodes.start, nodes.stop
                K = b_ - a
                SUB = (K + 15) // 16
                ks = [min(16, K - 16 * s) for s in range(SUB)]
                to_out = out_h is None
                dt_g = f32 if to_out else bf16
                have_f = has_l or has_r

                # x input: fp8 streamed tile, or resident bf16 tail slice
                if x8:
                    xt_t = inpool.tile([128, K, 4, BC], fp8, name="xt8_t")
                    nc.sync.dma_start(out=xt_t[:], in_=xt8_r[:, a:b_, :, :])
                    xv = xt_t
                else:
                    p0 = TAIL_POS[a]
                    xv = xtt_sb[:, p0 : p0 + K, :, :]

                def x_insts(s, j):
                    n0 = 16 * s
                    n1 = n0 + ks[s]
                    jx = j - 4 if j >= 16 else j  # fR's x-part is fx too
                    if x8:
                        i0 = WX_IDX[(0, jx)]
                        return [
                            (
                                wx8_sb[:, i0 + kk : i0 + kk + 2, :],
                                xv[:, n0:n1, kk : kk + 2, :].rearrange(
                                    "p n kt b -> p kt n b"
                                ),
                                DR,
                            )
                            for kk in (0, 2)
                        ]
                    return [
                        (
                            wx_sb[:, WX_IDX[(kk, jx)], :],
                            xv[:, n0:n1, kk, :],
                            None,
                        )
                        for kk in range(4)
                    ]

                def h_insts(s, j):
                    if child_h is None:
                        return []
                    ch_base = child_h[0][1]
                    n0, n1 = a + 16 * s, a + 16 * s + ks[s]
                    sl0 = 2 * n0 + 1 - ch_base
                    kk = n1 - n0

                    def nsl(off):
                        s0 = sl0 + off
                        if kk == 1:
                            return slice(s0, s0 + 1)
                        return slice(s0, s0 + 2 * kk - 1, 2)

                    sides = []
                    if has_l and j < 16:
                        sides.append(("L", 0))
                    if has_r and (j < 12 or 16 <= j):
                        sides.append(("R", 1))
                    out = []
                    for side, off in sides:
                        i0 = WH_IDX[(side, 0, j)]
                        for ct, _ in child_h:
                            for kta in (0, 2):
                                out.append(
                                    (
                                        wh8_sb[:, i0 + kta : i0 + kta + 2, :],
                                        ct[:, nsl(off), kta : kta + 2, :].rearrange(
                                            "p n kt b -> p kt n b"
                                        ),
                                        DR,
                                    )
                                )
                    return out

                def mm_group(ps, j, x_part=True, h_part=True):
                    # x phase for all subs first, then h phase: stalled
                    # h-matmuls sit behind ready x-work, not in front of it
                    # (PE dependency wait-queue is only 4 deep). Each sub's
                    # region is its own bank, so per-sub start flags are safe.
                    phases = []
                    for s in range(SUB):
                        xi = x_insts(s, j) if x_part else []
                        hi = h_insts(s, j) if h_part else []
                        phases.append((s, xi, hi))
                    for pi in range(2):
                        for s, xi, hi in phases:
                            psv = ps[:, s, : ks[s], :]
                            insts = xi if pi == 0 else hi
                            if not insts:
                                continue
                            first = pi == 0 or not xi
                            last = pi == 1 or not hi
                            for m, (w_ap, rhs, pm) in enumerate(insts):
                                kw = {} if pm is None else {"perf_mode": pm}
                                nc.tensor.matmul(
                                    psv,
                                    w_ap,
                                    rhs,
                                    start=(first and m == 0),
                                    stop=(last and m == len(insts) - 1),
                                    **kw,
                                )

                g_i = gpool.tile([128, K, 4, BC], dt_g, name="g_i", bufs=2)
                g_o = gpool.tile([128, K, 4, BC], dt_g, name="g_o", bufs=2)
                g_u = gpool.tile([128, K, 4, BC], dt_g, name="g_u", bufs=2)
                if has_l:
                    g_fl = gpool.tile([128, K, 4, BC], dt_g, name="g_fl")
                if has_r:
                    g_fr = gpool.tile([128, K, 4, BC], dt_g, name="g_fr")

                def gv(t, sl):
                    """[p, SUB, 16, b] view of gate tile t's output-slice sl."""
                    if SUB == 1:
                        return t[:, :K, sl, :]
                    return t[:].rearrange("p (s n) kt b -> p s n kt b", s=SUB)[
                        :, :, :, sl, :
                    ]

                def psv_all(ps):
                    if SUB == 1:
                        return ps[:, 0, :K, :]
                    return ps[:]

                if out_c is not None:
                    oc_t, oc_base = out_c
                    c_t = oc_t[:, a - oc_base : b_ - oc_base, :, :]
                else:
                    c_t = epool.tile([128, K, 4, BC], dt_g, name="c_t")[:]
                if to_out:
                    h_t = epool.tile([128, K, 4, BC], dt_g, name="h_t")

                if child_c is not None:
                    cc_t, cc_base = child_c
                    cs0 = 2 * a + 1 - cc_base
                    if has_l:
                        if K == 1:
                            cl_t = cc_t[:, cs0 : cs0 + 1, :, :]
                        else:
                            cl_t = cc_t[:, cs0 : cs0 + 2 * K - 1 : 2, :, :]
                    if has_r:
                        if K == 1:
                            cr_t = cc_t[:, cs0 + 1 : cs0 + 2, :, :]
                        else:
                            cr_t = cc_t[:, cs0 + 1 : cs0 + 2 * K : 2, :, :]

                def gate_act(dst, sl, j, func):
                    ps = pspool.tile([128, 2, 16, BC], f32, name="ps")
                    mm_group(ps, j)
                    nc.scalar.activation(
                        out=gv(dst, sl),
                        in_=psv_all(ps),
                        func=func,
                        bias=bias_sb[:, j : j + 1],
                        scale=1.0 / WSCALE,
                    )

                if K <= 8:
                    # ---- gate-merged tail path: one act per gate, bias
                    # injected into PSUM via identity-matmul with a
                    # stride-0 broadcast rhs (bias pre-scaled x16).
                    # Two-phase emission: ready bias/x matmuls for several
                    # groups first, stalled h matmuls after, so the 4-deep
                    # PE wait-queue never hides ready work. start= is set
                    # only on the first matmul of each PSUM bank (slices
                    # share banks at small K; start clears the whole bank's
                    # has_written bits). -------------------------------------
                    spb = max(1, 512 // (K * BC))  # slices per psum bank

                    def mm_tail(ps, j0, phase, bias_mm=True, x_part=True,
                                h_part=True):
                        for sl in range(4):
                            j = j0 + sl
                            a_insts = []
                            if bias_mm:
                                a_insts.append(
                                    (
                                        ident_sb[:],
                                        bias_s_sb[:, j : j + 1].broadcast_to(
                                            [128, K * BC]
                                        ),
                                        None,
                                    )
                                )
                            if x_part:
                                a_insts += x_insts(0, j)
                            b_insts = h_insts(0, j) if h_part else []
                            insts = a_insts if phase == 0 else b_insts
                            if not insts:
                                continue
                            first_of_slice = phase == 0 or not a_insts
                            last_of_slice = phase == 1 or not b_insts
                            for m, (w_ap, rhs, pm) in enumerate(insts):
                                kw = {} if pm is None else {"perf_mode": pm}
                                st = (
                                    first_of_slice and m == 0 and sl % spb == 0
                                )
                                sp = last_of_slice and m == len(insts) - 1
                                nc.tensor.matmul(
                                    ps[:, sl, :, :], w_ap, rhs,
                                    start=st, stop=sp, **kw,
                                )

                    def gview(t):
                        return t[:, :K, :, :].rearrange("p n s b -> p s n b")

                    def ps4():
                        return pspool.tile([128, 4, K, BC], f32, name="ps")

                    ps_fl = ps4() if has_l else None
                    ps_fr = ps4() if has_r else None
                    ps_i, ps_u = ps4(), ps4()
                    # phase A: all ready (bias + x) work
                    if has_l:
                        mm_tail(ps_fl, 12, 0)
                    if has_r:
                        mm_tail(ps_fr, 16, 0)
                    mm_tail(ps_i, 0, 0)
                    mm_tail(ps_u, 8, 0)
                    # phase B: h accumulation
                    if has_l:
                        mm_tail(ps_fl, 12, 1)
                    if has_r:
                        mm_tail(ps_fr, 16, 1)
                    mm_tail(ps_i, 0, 1)
                    mm_tail(ps_u, 8, 1)
                    nc.scalar.activation(
                        out=gview(g_i), in_=ps_i[:], func=ACT.Sigmoid,
                        scale=1.0 / WSCALE,
                    )
                    nc.scalar.activation(
                        out=gview(g_u), in_=ps_u[:], func=ACT.Tanh,
                        scale=1.0 / WSCALE,
                    )
                    if has_l:
                        nc.scalar.activation(
                            out=gview(g_fl), in_=ps_fl[:],
                            func=ACT.Sigmoid, scale=1.0 / WSCALE,
                        )
                    if has_r:
                        nc.scalar.activation(
                            out=gview(g_fr), in_=ps_fr[:],
                            func=ACT.Sigmoid, scale=1.0 / WSCALE,
                        )
                    ps_o = ps4()
                    mm_tail(ps_o, 4, 0)
                    mm_tail(ps_o, 4, 1)
                    nc.scalar.activation(
                        out=gview(g_o), in_=ps_o[:], func=ACT.Sigmoid,
                        scale=1.0 / WSCALE,
                    )

                    gi = g_i[:, :K, :, :]
                    go = g_o[:, :K, :, :]
                    gu = g_u[:, :K, :, :]
                    nc.vector.tensor_mul(c_t, gi, gu)
                    if has_l:
                        nc.vector.tensor_mul(gi, g_fl[:, :K, :, :], cl_t)
                        nc.vector.tensor_add(c_t, c_t, gi)
                    if has_r:
                        nc.vector.tensor_mul(gi, g_fr[:, :K, :, :], cr_t)
                        nc.vector.tensor_add(c_t, c_t, gi)
                    nc.scalar.activation(out=gu, in_=c_t, func=ACT.Tanh)
                    if to_out:
                        nc.vector.tensor_mul(h_t[:], go, gu)
                        nc.sync.dma_start(out=c0t_r[:], in_=c_t[:, 0, :, :])
                        nc.sync.dma_start(out=h0t_r[:], in_=h_t[:, 0, :, :])
                    elif len(out_h) == 1:
                        oh_t, oh_base = out_h[0]
                        nc.vector.tensor_mul(
                            oh_t[:, a - oh_base : b_ - oh_base, :, :], go, gu
                        )
                    else:
                        hbf = g_fl[:, :K, :, :]
                        nc.vector.tensor_mul(hbf, go, gu)
                        hsl = slice(a - out_h[0][1], b_ - out_h[0][1])
                        h8s = out_h[0][0][:, hsl, :, :]
                        nc.vector.tensor_copy(h8s, hbf)
                        nc.vector.tensor_sub(out_h[1][0][:, hsl, :, :], hbf, h8s)
                    return

                # two ktpair halves: groups emitted in consumption order
                # {i,u} -> {fx} -> {fL,fR} -> {o}, then this half's
                # elementwise + tanh + h, so DoubleRow consumers of child h
                # at the next level unblock per-ktpair.
                for hp in (0, 2):
                    sls = (hp, hp + 1)
                    for sl in sls:
                        gate_act(g_i, sl, sl, ACT.Sigmoid)
                        gate_act(g_u, sl, 8 + sl, ACT.Tanh)
                    if have_f:
                        # fx recomputed into each side's psum group (x_part);
                        # act reads psum directly with the fused bias.
                        for side_j, g_f in (
                            (12, g_fl if has_l else None),
                            (16, g_fr if has_r else None),
                        ):
                            if g_f is None:
                                continue
                            for sl in sls:
                                gate_act(g_f, sl, side_j + sl, ACT.Sigmoid)
                    for sl in sls:
                        gate_act(g_o, sl, 4 + sl, ACT.Sigmoid)

                    # --- elementwise for this ktpair half ----------------
                    h2 = slice(hp, hp + 2)
                    ch = c_t[:, :, h2, :]
                    gi = g_i[:, :K, h2, :]
                    go = g_o[:, :K, h2, :]
                    gu = g_u[:, :K, h2, :]
                    nc.vector.tensor_mul(ch, gi, gu)
                    if has_l:
                        nc.vector.tensor_mul(gi, g_fl[:, :K, h2, :], cl_t[:, :, h2, :])
                        nc.vector.tensor_add(ch, ch, gi)
                    if has_r:
                        nc.vector.tensor_mul(gi, g_fr[:, :K, h2, :], cr_t[:, :, h2, :])
                        nc.vector.tensor_add(ch, ch, gi)
                    # tanh(c) -> reuse g_u
                    nc.scalar.activation(out=gu, in_=ch, func=ACT.Tanh)
                    if to_out:
                        nc.vector.tensor_mul(h_t[:, :, h2, :], go, gu)
                    elif len(out_h) == 1:
                        oh_t, oh_base = out_h[0]
                        nc.vector.tensor_mul(
                            oh_t[:, a - oh_base : b_ - oh_base, h2, :], go, gu
                        )
                    else:
                        # split-h: hbf reuses g_fl (consumed above)
                        hbf = g_fl[:, :K, h2, :]
                        nc.vector.tensor_mul(hbf, go, gu)
                        hsl = slice(a - out_h[0][1], b_ - out_h[0][1])
                        h8s = out_h[0][0][:, hsl, h2, :]
                        nc.vector.tensor_copy(h8s, hbf)
                        nc.vector.tensor_sub(
                            out_h[1][0][:, hsl, h2, :], hbf, h8s
                        )

                if to_out:
                    nc.sync.dma_start(out=c0t_r[:], in_=c_t[:, 0, :, :])
                    nc.sync.dma_start(out=h0t_r[:], in_=h_t[:, 0, :, :])

            # h storage: plain fp8 levels 4..7; split fp8 pair levels 1..3.
            # c: fp8 at level 7, bf16 below.
            H_SPLIT_LVLS = (3, 2, 1)

            for _rep in range(reps):
                leafc_h = hpool.tile([128, 129, 4, BC], fp8, name="h_leafc")
                leafc_c = hpool.tile([128, 129, 4, BC], fp8, name="c_leafc")
                lvl_h = {7: [(leafc_h, 127)]}
                lvl_c = {7: (leafc_c, 127)}
                for lvl in range(6, 0, -1):
                    base = 2**lvl - 1
                    if lvl in H_SPLIT_LVLS:
                        t8 = hpool.tile([128, 2**lvl, 4, BC], fp8, name=f"h_{lvl}")
                        r8 = hpool.tile([128, 2**lvl, 4, BC], fp8, name=f"hr_{lvl}")
                        lvl_h[lvl] = [(t8, base), (r8, base)]
                    else:
                        t = hpool.tile([128, 2**lvl, 4, BC], fp8, name=f"h_{lvl}")
                        lvl_h[lvl] = [(t, base)]
                    t = hpool.tile([128, 2**lvl, 4, BC], bf16, name=f"c_{lvl}")
                    lvl_c[lvl] = (t, base)

                # leaves in 32-node super-chunks; the one holding node 255
                # first so node 127's serial chain hides behind the rest.
                for s4 in (224, 128):
                    process(
                        range(s4, s4 + 32), False, False, bleaf_sb, None,
                        lvl_h[7], out_c=lvl_c[7], x8=True,
                    )
                    if s4 == 224:
                        process(
                            range(127, 128), True, False, b1_sb, lvl_h[7],
                            lvl_h[7], child_c=lvl_c[7], out_c=lvl_c[7],
                            bias_s_sb=b1s_sb,
                        )
                for s4 in (160, 192):
                    process(
                        range(s4, s4 + 32), False, False, bleaf_sb, None,
                        lvl_h[7], out_c=lvl_c[7], x8=True,
                    )
                # L6: B-half (63..94, needs node 127 + leaves 128..190) after
                # A-half? A (95..126) needs leaves 191..254 -> do B first?
                # B needs 127..190 (ready after leaves 128..191); A needs
                # 191..254 (ready after all leaves). Emit B then A.
                process(
                    range(63, 95), True, True, b2_sb, lvl_h[7], lvl_h[6],
                    child_c=lvl_c[7], out_c=lvl_c[6], x8=True,
                )
                process(
                    range(95, 127), True, True, b2_sb, lvl_h[7], lvl_h[6],
                    child_c=lvl_c[7], out_c=lvl_c[6], x8=True,
                )
                # L5 (one 32-node super-chunk), then L4..L1
                process(
                    range(31, 63), True, True, b2_sb, lvl_h[6], lvl_h[5],
                    child_c=lvl_c[6], out_c=lvl_c[5], x8=True,
                )
                # L4 as two 8-node gate-merged chunks (pipeline each other)
                for a4 in (15, 23):
                    process(
                        range(a4, a4 + 8), True, True, b2_sb, lvl_h[5],
                        lvl_h[4], child_c=lvl_c[5], out_c=lvl_c[4],
                        bias_s_sb=b2s_sb,
                    )
                for lvl in range(3, 0, -1):
                    process(
                        range(2**lvl - 1, 2 ** (lvl + 1) - 1), True, True,
                        b2_sb, lvl_h[lvl + 1], lvl_h[lvl],
                        child_c=lvl_c[lvl + 1], out_c=lvl_c[lvl],
                        bias_s_sb=b2s_sb,
                    )
                process(
                    range(0, 1), True, True, b2_sb, lvl_h[1], None,
                    child_c=lvl_c[1], bias_s_sb=b2s_sb,
                )

    nc.compile()
    return nc


def _expected_tree():
    left = np.array([2 * i + 1 if 2 * i + 1 < N else 0 for i in range(N)], np.int32)
    right = np.array([2 * i + 2 if 2 * i + 2 < N else 0 for i in range(N)], np.int32)
    nch = np.array(
        [int(2 * i + 1 < N) + int(2 * i + 2 < N) for i in range(N)], np.int32
    )
    return left, right, nch


def pack_w(W_ioux, W_fx, W_iouhL, W_fhL, W_iouhR, W_fhR):
    """Returns (wx bf16, wx8 fp8, wh8 fp8), partition-major [128, blk, 128]."""
    s = WSCALE
    WxT = np.asarray(W_ioux, np.float32).T * s
    WfxT = np.asarray(W_fx, np.float32).T * s
    wx = np.empty((NWX, 128, 128), np.float32)
    for i, (kt, j) in enumerate(W_X_BLOCKS):
        src = WxT if j < 12 else WfxT
        jj = j if j < 12 else j - 12
        wx[i] = src[kt * 128 : (kt + 1) * 128, jj * 128 : (jj + 1) * 128]

    WhT = {
        "L": (np.asarray(W_iouhL, np.float32).T * s,
              np.asarray(W_fhL, np.float32).T * s),
        "R": (np.asarray(W_iouhR, np.float32).T * s,
              np.asarray(W_fhR, np.float32).T * s),
    }
    wh = np.empty((NWH, 128, 128), np.float32)
    for i, (side, kt, j) in enumerate(W_H_BLOCKS):
        iou_m, f_m = WhT[side]
        if j < 12:
            wh[i] = iou_m[kt * 128 : (kt + 1) * 128, j * 128 : (j + 1) * 128]
        else:
            jj = (j - 12) if j < 16 else (j - 16)
            wh[i] = f_m[kt * 128 : (kt + 1) * 128, jj * 128 : (jj + 1) * 128]

    wx_pm = np.ascontiguousarray(wx.transpose(1, 0, 2))  # [128, blk, 128]
    wh_pm = np.ascontiguousarray(wh.transpose(1, 0, 2))
    return wx_pm.astype(BF16), wx_pm.astype(FP8), wh_pm.astype(FP8)


def pack_biases(b_ioux, b_iouh, b_iouhL, b_iouhR, b_fx, b_fhL, b_fhR):
    def pack(vec):
        return np.ascontiguousarray(np.asarray(vec, np.float32).reshape(NJ, 128).T)

    z = np.zeros(512, np.float32)
    b2 = pack(np.concatenate([b_ioux + b_iouhL + b_iouhR, b_fx + b_fhL, b_fx + b_fhR]))
    bleaf = pack(np.concatenate([b_ioux + b_iouh, z, z]))
    b1 = pack(np.concatenate([b_ioux + b_iouhL, b_fx + b_fhL, z]))
    return b2, bleaf, b1


def pack_x_all(inputs):
    """inputs: [B, N, D] f32 -> per-core (xt8 [128,N,4,BC] fp8,
    xtt [128,32,4,BC] bf16) lists, one vectorized pass."""
    x = inputs.reshape(NCORES, BC, N, 4, 128)
    xt = np.ascontiguousarray(x.transpose(0, 4, 2, 3, 1))  # [C,128,N,4,BC]
    xt8 = xt.astype(FP8)
    tail = np.empty((NCORES, 128, 32, 4, BC), np.float32)
    tail[:, :, :31] = xt[:, :, :31]
    tail[:, :, 31] = xt[:, :, 127]
    tail = tail.astype(BF16)
    return [xt8[c] for c in range(NCORES)], [tail[c] for c in range(NCORES)]


class _Runner:
    """jit once per nc; reuse the executable across calls."""

    def __init__(self, nc, n_cores):
        import jax
        from concourse import bass2jax
        from concourse.bass2jax import _bass_exec_p, install_neuronx_cc_hook

        install_neuronx_cc_hook()
        self.nc = nc
        self.n_cores = n_cores
        partition_name = (
            nc.partition_id_tensor.name if nc.partition_id_tensor else None
        )
        in_names, out_names, out_avals, zero_outs = [], [], [], []
        for alloc in nc.m.functions[0].allocations:
            if not isinstance(alloc, mybir.MemoryLocationSet):
                continue
            name = alloc.memorylocations[0].name
            if alloc.kind == "ExternalInput":
                if name != partition_name:
                    in_names.append(name)
            elif alloc.kind == "ExternalOutput":
                out_names.append(name)
                shape = tuple(alloc.tensor_shape)
                dtype = mybir.dt.np(alloc.dtype)
                out_avals.append(jax.core.ShapedArray(shape, dtype))
                zero_outs.append(np.zeros(shape, dtype))
        self.in_names = in_names
        self.out_names = out_names
        self.zero_outs = zero_outs
        n_params = len(in_names)
        all_in = in_names + out_names
        if partition_name is not None:
            all_in.append(partition_name)

        def _body(*args):
            operands = list(args)
            if partition_name is not None:
                operands.append(bass2jax.partition_id_tensor())
            outs = _bass_exec_p.bind(
                *operands,
                out_avals=tuple(out_avals),
                in_names=tuple(all_in),
                out_names=tuple(out_names),
                lowering_input_output_aliases=(),
                sim_require_finite=True,
                sim_require_nnan=True,
                nc=nc,
            )
            return tuple(outs)

        if n_cores == 1:
            self.fn = jax.jit(_body, keep_unused=True)
        else:
            from jax.sharding import Mesh, PartitionSpec
            from jax.experimental.shard_map import shard_map

            devices = jax.devices()[:n_cores]
            mesh = Mesh(np.asarray(devices), ("core",))
            n_out = len(out_names)
            self.fn = jax.jit(
                shard_map(
                    _body,
                    mesh=mesh,
                    in_specs=(PartitionSpec("core"),) * (n_params + n_out),
                    out_specs=(PartitionSpec("core"),) * n_out,
                    check_rep=False,
                ),
                keep_unused=True,
            )

    def __call__(self, in_maps):
        import jax

        n = self.n_cores
        if n == 1:
            args = [np.asarray(in_maps[0][k]) for k in self.in_names]
            args += [np.zeros_like(z) for z in self.zero_outs]
            outs = self.fn(*args)
            jax.block_until_ready(outs)
            return [{k: np.asarray(outs[i]) for i, k in enumerate(self.out_names)}]
        args = [
            np.concatenate([np.asarray(m[k]) for m in in_maps], axis=0)
            for k in self.in_names
        ]
        args += [
            np.zeros((n * z.shape[0], *z.shape[1:]), z.dtype) for z in self.zero_outs
        ]
        outs = self.fn(*args)
        jax.block_until_ready(outs)
        res = []
        for c in range(n):
            d = {}
            for i, k in enumerate(self.out_names):
                full = np.asarray(outs[i])
                per = full.shape[0] // n
                d[k] = full[c * per : (c + 1) * per]
            res.append(d)
        return res


def _make_in_maps(inputs, weights_args):
    wx, wx8, wh8 = pack_w(*weights_args[:6])
    b2, bleaf, b1 = pack_biases(*weights_args[6:])
    inputs = np.asarray(inputs, np.float32)
    ident = np.eye(128, dtype=BF16)
    xt8s, xtts = pack_x_all(inputs)
    in_maps = []
    for c in range(NCORES):
        in_maps.append(
            {"xt8": xt8s[c], "xtt": xtts[c], "wx": wx, "wx8": wx8, "wh8": wh8,
             "b2": b2, "bleaf": bleaf, "b1": b1, "ident": ident}
        )
    return in_maps


def kernel(
    inputs,
    W_ioux, b_ioux, W_iouh, b_iouh, W_iouhL, b_iouhL, W_iouhR, b_iouhR,
    W_fx, b_fx, W_fh, b_fh, W_fhL, b_fhL, W_fhR, b_fhR,
    left_idx, right_idx, num_children,
):
    el, er, en = _expected_tree()
    assert np.array_equal(np.asarray(left_idx), el), "unexpected tree structure"
    assert np.array_equal(np.asarray(right_idx), er), "unexpected tree structure"
    assert np.array_equal(np.asarray(num_children), en), "unexpected tree structure"

    weights_args = (W_ioux, W_fx, W_iouhL, W_fhL, W_iouhR, W_fhR,
                    b_ioux, b_iouh, b_iouhL, b_iouhR, b_fx, b_fhL, b_fhR)
    in_maps = _make_in_maps(inputs, weights_args)

    if "nc" not in _compiled:
        _compiled["nc"] = _build_bass()
    nc = _compiled["nc"]
    if "runner" not in _compiled:
        _compiled["runner"] = _Runner(nc, NCORES)
    res = _compiled["runner"](in_maps)
    _compiled["last_res"] = res

    c_full = np.empty((B, D), np.float32)
    h_full = np.empty((B, D), np.float32)
    for c in range(NCORES):
        c_full[c * BC : (c + 1) * BC] = res[c]["c0t"].T
        h_full[c * BC : (c + 1) * BC] = res[c]["h0t"].T
    return c_full, h_full
